# revision 7
# baseline (speedup 1.0000x reference)
"""Trainium2 Bass kernel for the memristor-crossbar layer (nn_CustomLayer_30588757082254).

out = unmap(x @ G_eff) + bias, where G_eff = 1/(1/G + R_par) is an elementwise
transform of weight.T with globally min/max-normalized conductances.

Strategy: data-parallel over batch (8 cores x 1024 rows), single-phase N-split
schedule, fp16 streaming throughout (weights, x, transformed weights). fp16
halves SBUF traffic on every pass and on the PE's moving-operand stream --
SBUF port contention between the PE and DVE/ACT was the measured bottleneck
of the f32 variants -- while matmul cost stays 1 cycle/row and PSUM
accumulation stays fp32. All transform values lie in [-10.5, 10.5], far
inside fp16 range, and the 2^-11 rounding is ~20x under the error gate.

Math (S = 1/s folds the unmapping scale into the transform; kappa folded into
the transformed weights kills the row-sum correction; the parasitic term
c = s*R_par is separable, c = c0b[col] - rp[row], applied from a persistent
column tile + per-k-tile partition vector -- nothing streamed):
  s = (g_max-g_min)/(wmax-wmin);  a = g_min/s - wmin;  kappa = -a
  u  = WT + a                 (ACT, per-partition bias; WT streamed fp16)
  iu = 1/u                    (DVE fast recip, in-place)
  w  = (c0b - rp_kt) + iu     (DVE scalar_tensor_tensor, in-place)
  g0 = 1/w                    (DVE fast recip, in-place)
  ge = g0 + kappa             (ACT bias add -> fp16)   [ge = S*G_eff + kappa]
  out = x @ ge + bias         (PE; bias added into PSUM via ones-row matmul)

Schedule per core: transform streams 32 half-tiles [128,1024] (2 col-halves x
16 k-tiles); all ge tiles stay resident so matmul pacing never recycles them.
Matmuls consume ge in 4 column-quarters of 512 (one PSUM bank per (quarter,
m-block), 8 banks live), kt-outer / mb-inner, bias matmul closes each group,
ACT copy drains PSUM for the output DMA. Emission order
[T h0 + x stream][q0][T h1][q1][q2][q3]. Host prep is layout + fp16 cast +
scalar weight stats + index-derived parasitic vectors.
"""
import numpy as np

import concourse.bass as bass
import concourse.mybir as mybir
import concourse.tile as tile
from concourse import bacc
from concourse.bass_utils import run_bass_kernel_spmd
from concourse.dve_ops import RECIP_APPROX_FAST_CONSTS, RECIPROCAL_APPROX_FAST

F32 = mybir.dt.float32
F32R = mybir.dt.float32r
F16 = mybir.dt.float16
AF = mybir.ActivationFunctionType
ALU = mybir.AluOpType
CRC = RECIP_APPROX_FAST_CONSTS

N_CORES = 8
B, K, N = 8192, 2048, 2048
BC = B // N_CORES            # 1024 batch rows per core
KT = K // 128                # 16 k-tiles
MB = BC // 128               # 8 m-blocks per core
NH = 2                       # transform col-halves of 1024

PARASITIC_R = 2.0
G_MIN, G_MAX = 1.0 / 100000.0, 1.0 / 1000.0

_CACHE = {}


def _build_nc():
    nc = bacc.Bacc("TRN2", target_bir_lowering=False, debug=False,
                   num_devices=N_CORES)
    # wt blocked as [h, kt] -> [128, 1024] tiles, rows contiguous per tile
    wt_in = nc.dram_tensor("wt", [NH * KT * 128, 1024], F16,
                           kind="ExternalInput")
    xt_in = nc.dram_tensor("xt", [128, KT * BC], F16, kind="ExternalInput")
    bias_in = nc.dram_tensor("bias", [1, N], F16, kind="ExternalInput")
    mmx_in = nc.dram_tensor("mmx", [128, 4], F32, kind="ExternalInput")
    cf_in = nc.dram_tensor("cf", [NH * KT * 128, 1024], F16,
                           kind="ExternalInput")
    out_d = nc.dram_tensor("out", [BC, N], F32, kind="ExternalOutput")

    with tile.TileContext(nc) as tc:
        with (
            tc.tile_pool(name="wtp", bufs=4) as wtp,
            tc.tile_pool(name="cfp", bufs=4) as cfp,
            tc.tile_pool(name="up", bufs=6) as up,
            tc.tile_pool(name="gep", bufs=NH * KT + 2) as gep,
            tc.tile_pool(name="xtp", bufs=KT) as xtp,
            tc.tile_pool(name="osbp", bufs=4) as osbp,
            tc.tile_pool(name="smallp", bufs=1) as sp,
            tc.tile_pool(name="pcp", bufs=8, space="PSUM") as pcp,
        ):
            # ---------------- tiny inputs ----------------
            with nc.named_scope("setup"):
                bcv = sp.tile([128, 4], F32, tag="bcv")
                nc.sync.dma_start(out=bcv[:], in_=mmx_in[:])
                bias_row = sp.tile([1, N], F16, tag="bias_row")
                nc.sync.dma_start(out=bias_row[:], in_=bias_in[:])
                ones_row_f = sp.tile([1, 128], F32, tag="ones_row_f")
                nc.vector.memset(ones_row_f[:], 1.0)
                ones_row = sp.tile([1, 128], F16, tag="ones_row")
                nc.vector.tensor_copy(ones_row[:], ones_row_f[:])
                # trigger the lazy ACT table load before real work arrives
                warm = sp.tile([1, 2], F32, tag="warm")
                nc.vector.memset(warm[:], 0.0)
                nc.scalar.activation(warm[:], warm[:], AF.Identity,
                                     bias=0.0, scale=1.0)
            a_b = bcv[:, 0:1]
            kap_b = bcv[:, 1:2]

            xt_t = {}
            ge = {}

            def transform_half(h):
                """ge[h, kt] = S*G_eff + kappa, fp16 tiles [128, 1024]."""
                for kt in range(KT):
                    r0 = (h * KT + kt) * 128
                    w_t = wtp.tile([128, 1024], F16, tag="wt",
                                   name=f"wt{h}_{kt}")
                    nc.sync.dma_start(out=w_t[:], in_=wt_in[r0:r0 + 128, :])
                    c_t = cfp.tile([128, 1024], F16, tag="cf",
                                   name=f"cf{h}_{kt}")
                    nc.scalar.dma_start(out=c_t[:], in_=cf_in[r0:r0 + 128, :])
                    if h == 0:
                        # interleave x slices so they trail wt per k-tile
                        x_t = xtp.tile([128, BC], F16, tag="xt",
                                       name=f"xt{kt}")
                        nc.gpsimd.dma_start(
                            out=x_t[:], in_=xt_in[:, kt * BC:(kt + 1) * BC])
                        xt_t[kt] = x_t
                    # u = WT + a
                    u_t = up.tile([128, 1024], F16, tag="u", name=f"u{h}_{kt}")
                    nc.scalar.activation(u_t[:], w_t[:], AF.Identity,
                                         bias=a_b, scale=1.0)
                    # iu = 1/u (in-place)
                    nc.vector._custom_dve(RECIPROCAL_APPROX_FAST, out=u_t[:],
                                          in0=u_t[:], s0=CRC["s0"],
                                          s1=CRC["s1"], imm2=CRC["imm2"])
                    # w = c + iu (GpSimd, in-place)
                    nc.gpsimd.tensor_tensor(u_t[:], u_t[:], c_t[:], ALU.add)
                    # g0 = 1/w (in-place)
                    nc.vector._custom_dve(RECIPROCAL_APPROX_FAST, out=u_t[:],
                                          in0=u_t[:], s0=CRC["s0"],
                                          s1=CRC["s1"], imm2=CRC["imm2"])
                    # ge = g0 + kappa -> fp16
                    g_t = gep.tile([128, 1024], F16, tag="ge",
                                   name=f"ge{h}_{kt}")
                    nc.scalar.activation(g_t[:], u_t[:], AF.Identity,
                                         bias=kap_b, scale=1.0)
                    ge[h, kt] = g_t

            def quarter_sweep(q):
                h, qq = q // 2, q % 2
                cs = qq * 512
                pcs = {}
                for kt in range(KT):
                    for mb in range(MB):
                        if kt == 0:
                            pcs[mb] = pcp.tile([128, 512], F32, tag="pc",
                                               name=f"p{q}_{mb}")
                        nc.tensor.matmul(
                            pcs[mb][:],
                            xt_t[kt][:, mb * 128:(mb + 1) * 128],
                            ge[h, kt][:, cs:cs + 512],
                            start=(kt == 0), stop=False)
                for mb in range(MB):
                    nc.tensor.matmul(pcs[mb][:], ones_row[:],
                                     bias_row[:, q * 512:(q + 1) * 512],
                                     start=False, stop=True)
                    osb = osbp.tile([128, 512], F32, tag="osb",
                                    name=f"o{q}_{mb}")
                    nc.scalar.copy(osb[:], pcs[mb][:])
                    nc.scalar.dma_start(
                        out=out_d[mb * 128:(mb + 1) * 128,
                                  q * 512:(q + 1) * 512],
                        in_=osb[:])

            with nc.named_scope("t_h0"):
                transform_half(0)
            with nc.named_scope("q0"):
                quarter_sweep(0)
            with nc.named_scope("t_h1"):
                transform_half(1)
            with nc.named_scope("q1"):
                quarter_sweep(1)
            with nc.named_scope("q2"):
                quarter_sweep(2)
            with nc.named_scope("q3"):
                quarter_sweep(3)
    nc.finalize()
    return nc


def _prep_inputs(x, weight, bias):
    wtT = np.ascontiguousarray(weight.T)          # [K, N]
    wmin = float(wtT.min())
    wmax = float(wtT.max())
    s = (G_MAX - G_MIN) / (wmax - wmin)
    a = G_MIN / s - wmin
    kappa = wmin - G_MIN / s
    mmx = np.zeros((128, 4), dtype=np.float32)
    mmx[:, 0] = a
    mmx[:, 1] = kappa
    # full parasitic matrix c = s*(4098 + 2n - 2*row), blocked like wt, fp16
    row = np.arange(K, dtype=np.float64)[:, None]
    coln = np.arange(N, dtype=np.float64)[None, :]
    cf = (np.float64(s) * (4098.0 + 2.0 * coln - 2.0 * row)).astype(np.float16)
    cf_b = np.ascontiguousarray(
        cf.reshape(KT, 128, NH, 1024).transpose(2, 0, 1, 3)
        .reshape(NH * KT * 128, 1024))

    # wt blocked [h, kt] -> [128, 1024], fp16
    wt_b = np.ascontiguousarray(
        wtT.reshape(KT, 128, NH, 1024).transpose(2, 0, 1, 3)
        .reshape(NH * KT * 128, 1024).astype(np.float16))

    bias2 = np.ascontiguousarray(bias.reshape(1, N)).astype(np.float16)
    in_maps = []
    for c in range(N_CORES):
        x_c = x[c * BC:(c + 1) * BC, :]           # [BC, K]
        # xh[p, kt, m] = x_c[m, kt*128+p]
        xh = np.ascontiguousarray(
            x_c.T.reshape(KT, 128, BC).transpose(1, 0, 2)
            .reshape(128, KT * BC).astype(np.float16))
        in_maps.append({"wt": wt_b, "xt": xh, "bias": bias2, "mmx": mmx,
                        "cf": cf_b})
    return in_maps


def _run(x, weight, bias, trace=False, trace_kwargs=None):
    if "nc" not in _CACHE:
        _CACHE["nc"] = _build_nc()
    nc = _CACHE["nc"]
    in_maps = _prep_inputs(x, weight, bias)
    res = run_bass_kernel_spmd(nc, in_maps, list(range(N_CORES)), trace=trace,
                               **(trace_kwargs or {}))
    out = np.concatenate([res.results[c]["out"] for c in range(N_CORES)], axis=0)
    return out, res


def kernel(x, weight, bias):
    x = np.asarray(x, dtype=np.float32)
    weight = np.asarray(weight, dtype=np.float32)
    bias = np.asarray(bias, dtype=np.float32)
    out, _ = _run(x, weight, bias, trace=False)
    return out.astype(np.float32)


# revision 8
# speedup vs baseline: 1.2436x; 1.2436x over previous
"""Trainium2 Bass kernel for the memristor-crossbar layer (nn_CustomLayer_30588757082254).

out = unmap(x @ G_eff) + bias, where G_eff = 1/(1/G + R_par) is an elementwise
transform of weight.T with globally min/max-normalized conductances.

Strategy: data-parallel over batch (8 cores x 1024 rows), single-phase N-split
schedule, fp16 streaming throughout (weights, x, transformed weights). fp16
halves SBUF traffic on every pass and on the PE's moving-operand stream --
SBUF port contention between the PE and DVE/ACT was the measured bottleneck
of the f32 variants -- while matmul cost stays 1 cycle/row and PSUM
accumulation stays fp32. All transform values lie in [-10.5, 10.5], far
inside fp16 range, and the 2^-11 rounding is ~20x under the error gate.

Math (S = 1/s folds the unmapping scale into the transform; kappa folded into
the transformed weights kills the row-sum correction; the parasitic term
c = s*R_par is separable, c = c0b[col] - rp[row], applied from a persistent
column tile + per-k-tile partition vector -- nothing streamed):
  s = (g_max-g_min)/(wmax-wmin);  a = g_min/s - wmin;  kappa = -a
  u  = WT + a                 (ACT, per-partition bias; WT streamed fp16)
  iu = 1/u                    (DVE fast recip, in-place)
  w  = (c0b - rp_kt) + iu     (DVE scalar_tensor_tensor, in-place)
  g0 = 1/w                    (DVE fast recip, in-place)
  ge = g0 + kappa             (ACT bias add -> fp16)   [ge = S*G_eff + kappa]
  out = x @ ge + bias         (PE; bias added into PSUM via ones-row matmul)

Schedule per core: transform streams 32 half-tiles [128,1024] (2 col-halves x
16 k-tiles); all ge tiles stay resident so matmul pacing never recycles them.
Matmuls consume ge in 4 column-quarters of 512 (one PSUM bank per (quarter,
m-block), 8 banks live), kt-outer / mb-inner, bias matmul closes each group,
ACT copy drains PSUM for the output DMA. Emission order
[T h0 + x stream][q0][T h1][q1][q2][q3]. Host prep is layout + fp16 cast +
scalar weight stats + index-derived parasitic vectors.
"""
import numpy as np

import concourse.bass as bass
import concourse.mybir as mybir
import concourse.tile as tile
from concourse import bacc
from concourse.bass_utils import run_bass_kernel_spmd
from concourse.dve_ops import RECIP_APPROX_FAST_CONSTS, RECIPROCAL_APPROX_FAST

F32 = mybir.dt.float32
F32R = mybir.dt.float32r
F16 = mybir.dt.float16
AF = mybir.ActivationFunctionType
ALU = mybir.AluOpType
CRC = RECIP_APPROX_FAST_CONSTS

N_CORES = 8
B, K, N = 8192, 2048, 2048
BC = B // N_CORES            # 1024 batch rows per core
KT = K // 128                # 16 k-tiles
MB = BC // 128               # 8 m-blocks per core
NH = 2                       # transform col-halves of 1024

PARASITIC_R = 2.0
G_MIN, G_MAX = 1.0 / 100000.0, 1.0 / 1000.0

_CACHE = {}


def _build_nc():
    nc = bacc.Bacc("TRN2", target_bir_lowering=False, debug=False,
                   num_devices=N_CORES)
    # wt blocked as [h, kt] -> [128, 1024] tiles, rows contiguous per tile
    wt_in = nc.dram_tensor("wt", [NH * KT * 128, 1024], F16,
                           kind="ExternalInput")
    xt_in = nc.dram_tensor("xt", [128, KT * BC], F16, kind="ExternalInput")
    bias_in = nc.dram_tensor("bias", [1, N], F16, kind="ExternalInput")
    mmx_in = nc.dram_tensor("mmx", [128, 4], F32, kind="ExternalInput")
    rp_in = nc.dram_tensor("rp", [128, KT], F16, kind="ExternalInput")
    c0_in = nc.dram_tensor("c0", [128, N], F16, kind="ExternalInput")
    out_d = nc.dram_tensor("out", [BC, N], F32, kind="ExternalOutput")

    with tile.TileContext(nc) as tc:
        with (
            tc.tile_pool(name="wtp", bufs=4) as wtp,
            tc.tile_pool(name="up", bufs=6) as up,
            tc.tile_pool(name="gep", bufs=NH * KT + 2) as gep,
            tc.tile_pool(name="xtp", bufs=KT) as xtp,
            tc.tile_pool(name="osbp", bufs=4) as osbp,
            tc.tile_pool(name="smallp", bufs=1) as sp,
            tc.tile_pool(name="pcp", bufs=8, space="PSUM") as pcp,
        ):
            # ---------------- tiny inputs ----------------
            with nc.named_scope("setup"):
                bcv = sp.tile([128, 4], F32, tag="bcv")
                nc.sync.dma_start(out=bcv[:], in_=mmx_in[:])
                rpn = sp.tile([128, KT], F16, tag="rpn")
                nc.sync.dma_start(out=rpn[:], in_=rp_in[:])
                bias_row = sp.tile([1, N], F16, tag="bias_row")
                nc.sync.dma_start(out=bias_row[:], in_=bias_in[:])
                ones_row_f = sp.tile([1, 128], F32, tag="ones_row_f")
                nc.vector.memset(ones_row_f[:], 1.0)
                ones_row = sp.tile([1, 128], F16, tag="ones_row")
                nc.vector.tensor_copy(ones_row[:], ones_row_f[:])
                # trigger the lazy ACT table load before real work arrives
                warm = sp.tile([1, 2], F32, tag="warm")
                nc.vector.memset(warm[:], 0.0)
                nc.scalar.activation(warm[:], warm[:], AF.Identity,
                                     bias=0.0, scale=1.0)
            a_b = bcv[:, 0:1]
            kap_b = bcv[:, 1:2]

            xt_t = {}
            ge = {}
            c0b = sp.tile([128, N], F16, tag="c0b")

            def transform_half(h):
                """ge[h, kt] = S*G_eff + kappa, fp16 tiles [128, 1024]."""
                for kt in range(KT):
                    r0 = (h * KT + kt) * 128
                    w_t = wtp.tile([128, 1024], F16, tag="wt",
                                   name=f"wt{h}_{kt}")
                    nc.sync.dma_start(out=w_t[:], in_=wt_in[r0:r0 + 128, :])
                    if h == 0:
                        # interleave x slices so they trail wt per k-tile
                        x_t = xtp.tile([128, BC], F16, tag="xt",
                                       name=f"xt{kt}")
                        nc.sync.dma_start(
                            out=x_t[:], in_=xt_in[:, kt * BC:(kt + 1) * BC])
                        xt_t[kt] = x_t
                        if kt == 0:
                            # bulky setup tile rides behind the first k-tile
                            nc.sync.dma_start(out=c0b[:], in_=c0_in[:])
                    # u = WT + a
                    u_t = up.tile([128, 1024], F16, tag="u", name=f"u{h}_{kt}")
                    nc.scalar.activation(u_t[:], w_t[:], AF.Identity,
                                         bias=a_b, scale=1.0)
                    # iu = 1/u (in-place)
                    nc.vector._custom_dve(RECIPROCAL_APPROX_FAST, out=u_t[:],
                                          in0=u_t[:], s0=CRC["s0"],
                                          s1=CRC["s1"], imm2=CRC["imm2"])
                    # w = (c0b - rp_kt) + iu (in-place)
                    nc.vector.scalar_tensor_tensor(
                        u_t[:], c0b[:, h * 1024:(h + 1) * 1024],
                        rpn[:, kt:kt + 1], u_t[:], ALU.add, ALU.add)
                    # g0 = 1/w (in-place)
                    nc.vector._custom_dve(RECIPROCAL_APPROX_FAST, out=u_t[:],
                                          in0=u_t[:], s0=CRC["s0"],
                                          s1=CRC["s1"], imm2=CRC["imm2"])
                    # ge = g0 + kappa -> fp16
                    g_t = gep.tile([128, 1024], F16, tag="ge",
                                   name=f"ge{h}_{kt}")
                    nc.scalar.activation(g_t[:], u_t[:], AF.Identity,
                                         bias=kap_b, scale=1.0)
                    ge[h, kt] = g_t

            def quarter_sweep(q):
                h, qq = q // 2, q % 2
                cs = qq * 512
                pcs = {}
                for kt in range(KT):
                    for mb in range(MB):
                        if kt == 0:
                            pcs[mb] = pcp.tile([128, 512], F32, tag="pc",
                                               name=f"p{q}_{mb}")
                        nc.tensor.matmul(
                            pcs[mb][:],
                            xt_t[kt][:, mb * 128:(mb + 1) * 128],
                            ge[h, kt][:, cs:cs + 512],
                            start=(kt == 0), stop=False)
                for mb in range(MB):
                    nc.tensor.matmul(pcs[mb][:], ones_row[:],
                                     bias_row[:, q * 512:(q + 1) * 512],
                                     start=False, stop=True)
                    osb = osbp.tile([128, 512], F32, tag="osb",
                                    name=f"o{q}_{mb}")
                    nc.scalar.copy(osb[:], pcs[mb][:])
                    nc.scalar.dma_start(
                        out=out_d[mb * 128:(mb + 1) * 128,
                                  q * 512:(q + 1) * 512],
                        in_=osb[:])

            with nc.named_scope("t_h0"):
                transform_half(0)
            with nc.named_scope("q0"):
                quarter_sweep(0)
            with nc.named_scope("t_h1"):
                transform_half(1)
            with nc.named_scope("q1"):
                quarter_sweep(1)
            with nc.named_scope("q2"):
                quarter_sweep(2)
            with nc.named_scope("q3"):
                quarter_sweep(3)
    nc.finalize()
    return nc


def _prep_inputs(x, weight, bias):
    wtT = np.ascontiguousarray(weight.T)          # [K, N]
    wmin = float(wtT.min())
    wmax = float(wtT.max())
    s = (G_MAX - G_MIN) / (wmax - wmin)
    a = G_MIN / s - wmin
    kappa = wmin - G_MIN / s
    p_idx = np.arange(128, dtype=np.float64)
    mmx = np.zeros((128, 4), dtype=np.float32)
    mmx[:, 0] = a
    mmx[:, 1] = kappa
    # rp[p, kt] = -2*s*(128*kt + p)  (so c = c0b + rp)
    rp = np.zeros((128, KT), dtype=np.float16)
    for kt in range(KT):
        rp[:, kt] = (-2.0 * s * (128.0 * kt + p_idx)).astype(np.float16)
    # c0b[p, n] = s*(4098 + 2n), same for all partitions
    coln = np.arange(N, dtype=np.float64)[None, :]
    c0 = np.ascontiguousarray(np.broadcast_to(
        (np.float64(s) * (4098.0 + 2.0 * coln)).astype(np.float16), (128, N)))

    # wt blocked [h, kt] -> [128, 1024], fp16
    wt_b = np.ascontiguousarray(
        wtT.reshape(KT, 128, NH, 1024).transpose(2, 0, 1, 3)
        .reshape(NH * KT * 128, 1024).astype(np.float16))

    bias2 = np.ascontiguousarray(bias.reshape(1, N)).astype(np.float16)
    in_maps = []
    for c in range(N_CORES):
        x_c = x[c * BC:(c + 1) * BC, :]           # [BC, K]
        # xh[p, kt, m] = x_c[m, kt*128+p]
        xh = np.ascontiguousarray(
            x_c.T.reshape(KT, 128, BC).transpose(1, 0, 2)
            .reshape(128, KT * BC).astype(np.float16))
        in_maps.append({"wt": wt_b, "xt": xh, "bias": bias2, "mmx": mmx,
                        "rp": rp, "c0": c0})
    return in_maps


def _run(x, weight, bias, trace=False, trace_kwargs=None):
    if "nc" not in _CACHE:
        _CACHE["nc"] = _build_nc()
    nc = _CACHE["nc"]
    in_maps = _prep_inputs(x, weight, bias)
    res = run_bass_kernel_spmd(nc, in_maps, list(range(N_CORES)), trace=trace,
                               **(trace_kwargs or {}))
    out = np.concatenate([res.results[c]["out"] for c in range(N_CORES)], axis=0)
    return out, res


def kernel(x, weight, bias):
    x = np.asarray(x, dtype=np.float32)
    weight = np.asarray(weight, dtype=np.float32)
    bias = np.asarray(bias, dtype=np.float32)
    out, _ = _run(x, weight, bias, trace=False)
    return out.astype(np.float32)
